# revision 20
# baseline (speedup 1.0000x reference)
import numpy as np
import concourse.bacc as bacc
import concourse.mybir as mybir
from concourse.tile import TileContext
from concourse.bass_utils import run_bass_kernel_spmd

DIM_INPUT = 128
DIM_REC = 512
DIM_OUT = 256
BATCH = 512
NCORES = 8
B = BATCH // NCORES  # 64 per-core batch
T = DIM_INPUT        # 128 timesteps
KJ = DIM_REC // 128  # 4 chunks of the recurrent dim
OJ = DIM_OUT // 128  # 2 chunks of the output dim

F32 = mybir.dt.float32
MMDT = mybir.dt.float16  # matmul operand dtype (FWL + 1 cyc/row on PE)
MMNP = np.float16

# MM issue order within a step. Each output group j accumulates 5 MMs:
# an x-projection MM ('x', start=True: psum = x @ Wx[j]) plus 4 recurrent
# MMs (k=0..3). Recomputing the x MM each step seeds psum so the epilogue
# is a single fused bias+relu per group. The (j,k) suborder maximizes the
# min slack between group-k completion and the next step's first consumer
# of g'_k (slack 12 of 20 slots; >=13 provably infeasible).
STEP_ORDER = [
    (0, 'x'), (1, 'x'), (2, 'x'), (3, 'x'),
    (0, 0), (1, 0), (2, 0), (3, 0),
    (0, 1), (1, 1), (2, 1),
    (0, 2), (0, 3),
    (1, 2), (1, 3),
    (3, 1),
    (2, 2), (2, 3),
    (3, 2), (3, 3),
]


def _build_nc():
    nc = bacc.Bacc("TRN2", target_bir_lowering=False, debug=False,
                   num_devices=NCORES)
    xT = nc.dram_tensor("xT", [DIM_INPUT, B], MMDT, kind="ExternalInput")
    WhT = nc.dram_tensor("WhT", [DIM_REC, DIM_REC], MMDT, kind="ExternalInput")
    WxT = nc.dram_tensor("WxT", [DIM_INPUT, DIM_REC], MMDT, kind="ExternalInput")
    WhyT = nc.dram_tensor("WhyT", [DIM_REC, DIM_OUT], MMDT, kind="ExternalInput")
    bc = nc.dram_tensor("bc", [DIM_REC, 1], F32, kind="ExternalInput")
    by = nc.dram_tensor("by", [DIM_OUT, 1], F32, kind="ExternalInput")
    yT = nc.dram_tensor("yT", [DIM_OUT, B], F32, kind="ExternalOutput")

    RELU = mybir.ActivationFunctionType.Relu
    IDENT = mybir.ActivationFunctionType.Identity

    with TileContext(nc) as tc:
        with tc.tile_pool(name="w", bufs=1) as wp, \
             tc.tile_pool(name="s", bufs=1) as sp, \
             tc.psum_pool(name="p", bufs=1) as pp:
            wh = [wp.tile([128, DIM_REC], MMDT, name=f"wh{k}") for k in range(KJ)]
            wx = wp.tile([128, DIM_REC], MMDT, name="wx")
            why = [wp.tile([128, DIM_OUT], MMDT, name=f"why{k}") for k in range(KJ)]
            bct = [wp.tile([128, 1], F32, name=f"bct{k}") for k in range(KJ)]
            byt = [wp.tile([128, 1], F32, name=f"byt{j}") for j in range(OJ)]
            xt = sp.tile([128, B], MMDT, name="xt")
            g = [[sp.tile([128, B], MMDT, name=f"g{p}_{k}") for k in range(KJ)]
                 for p in range(2)]
            ps = [[pp.tile([128, B], F32, name=f"ps{p}_{j}") for j in range(KJ)]
                  for p in range(2)]
            psy = [ps[0][0], ps[0][1]]  # reuse phase-0 banks (free after step T-1)

            # startup DMAs spread across engine queues so the big weight
            # loads run in parallel; why/byt are only needed after the loop
            # so they trail on the sync queue.
            nc.sync.dma_start(out=xt[:], in_=xT[:])
            nc.sync.dma_start(out=wx[:], in_=WxT[:])
            for k in range(KJ):
                nc.sync.dma_start(out=bct[k][:], in_=bc[k * 128:(k + 1) * 128, :])
            nc.gpsimd.dma_start(out=wh[0][:], in_=WhT[0:128, :])
            nc.scalar.dma_start(out=wh[1][:], in_=WhT[128:256, :])
            nc.gpsimd.dma_start(out=wh[2][:], in_=WhT[256:384, :])
            nc.scalar.dma_start(out=wh[3][:], in_=WhT[384:512, :])
            for k in range(KJ):
                nc.sync.dma_start(out=why[k][:], in_=WhyT[k * 128:(k + 1) * 128, :])
            for j in range(OJ):
                nc.sync.dma_start(out=byt[j][:], in_=by[j * 128:(j + 1) * 128, :])

            ADD = mybir.AluOpType.add
            MAX = mybir.AluOpType.max

            def epilogue(dst, psrc):
                # dst_j = relu(psum_j + bc_j). Pairing (0,2) on ScalarE and
                # (1,3) on DVE staggers each engine's two ops so the second
                # op's queue wait does not extend the critical relu chain.
                nc.scalar.activation(dst[0][:], psrc[0][:], RELU,
                                     bias=bct[0][:])
                nc.vector.tensor_scalar(dst[1][:], psrc[1][:],
                                        bct[1][:], 0.0, ADD, MAX)
                nc.scalar.activation(dst[2][:], psrc[2][:], RELU,
                                     bias=bct[2][:])
                nc.vector.tensor_scalar(dst[3][:], psrc[3][:],
                                        bct[3][:], 0.0, ADD, MAX)

            # step 1 (h0 = 0): g0_j = relu((x @ W_x2h.T).T[j] + bc[j])
            for j in range(KJ):
                nc.tensor.matmul(ps[0][j][:], wx[:, j * 128:(j + 1) * 128],
                                 xt[:], start=True, stop=True)
            epilogue(g[0], ps[0])

            # 127 recurrent steps: g' = relu(x @ Wx + Wh @ g + bc)
            for s in range(1, T):
                cur, nxt = g[(s + 1) % 2], g[s % 2]
                pcur = ps[s % 2]
                grp = [0] * KJ
                for (j, k) in STEP_ORDER:
                    if k == 'x':
                        nc.tensor.matmul(pcur[j][:],
                                         wx[:, j * 128:(j + 1) * 128],
                                         xt[:], start=True, stop=False)
                    else:
                        nc.tensor.matmul(pcur[j][:],
                                         wh[k][:, j * 128:(j + 1) * 128],
                                         cur[k][:], start=False,
                                         stop=(grp[j] == KJ - 1))
                        grp[j] += 1
                epilogue(nxt, pcur)

            gfin = g[(T - 1) % 2]
            # yT[jslice] = W_h2y[jslice] @ h.T + b_h2y[jslice]
            for j in range(OJ):
                for k in range(KJ):
                    nc.tensor.matmul(psy[j][:], why[k][:, j * 128:(j + 1) * 128],
                                     gfin[k][:], start=(k == 0), stop=(k == KJ - 1))
            ytile = [sp.tile([128, B], F32, name=f"yt{j}") for j in range(OJ)]
            nc.scalar.activation(ytile[0][:], psy[0][:], IDENT, bias=byt[0][:])
            nc.vector.tensor_scalar(ytile[1][:], psy[1][:], byt[1][:], None, ADD)
            nc.sync.dma_start(out=yT[0:128, :], in_=ytile[0][:])
            nc.gpsimd.dma_start(out=yT[128:256, :], in_=ytile[1][:])

    nc.compile()
    return nc


_NC = None
TRACE = False
TRACE_TMPDIR = None
LAST_RESULTS = None


def kernel(x, W_x2h, b_x2h, W_h2h, b_h2h, W_h2y, b_h2y):
    global _NC, LAST_RESULTS
    if _NC is None:
        _NC = _build_nc()

    x = np.asarray(x, np.float32)
    shared = {
        "WhT": np.ascontiguousarray(np.asarray(W_h2h, np.float32).T.astype(MMNP)),
        "WxT": np.ascontiguousarray(np.asarray(W_x2h, np.float32).T.astype(MMNP)),
        "WhyT": np.ascontiguousarray(np.asarray(W_h2y, np.float32).T.astype(MMNP)),
        "bc": (np.asarray(b_x2h, np.float32)
               + np.asarray(b_h2h, np.float32)).reshape(DIM_REC, 1),
        "by": np.asarray(b_h2y, np.float32).reshape(DIM_OUT, 1),
    }
    ins = []
    for i in range(NCORES):
        m = dict(shared)
        m["xT"] = np.ascontiguousarray(x[i * B:(i + 1) * B, :].T.astype(MMNP))
        ins.append(m)

    kw = {}
    if TRACE:
        kw = {"trace": True, "tmpdir": TRACE_TMPDIR}
    res = run_bass_kernel_spmd(_NC, ins, core_ids=list(range(NCORES)), **kw)
    LAST_RESULTS = res
    out = np.empty((BATCH, DIM_OUT), np.float32)
    for i in range(NCORES):
        out[i * B:(i + 1) * B, :] = res.results[i]["yT"].T
    return out


# revision 21
# speedup vs baseline: 1.1861x; 1.1861x over previous
import numpy as np
import concourse.bacc as bacc
import concourse.mybir as mybir
from concourse.tile import TileContext
from concourse.bass_utils import run_bass_kernel_spmd

DIM_INPUT = 128
DIM_REC = 512
DIM_OUT = 256
BATCH = 512
NCORES = 8
B = BATCH // NCORES  # 64 per-core batch
T = DIM_INPUT        # 128 timesteps
KJ = DIM_REC // 128  # 4 chunks of the recurrent dim
OJ = DIM_OUT // 128  # 2 chunks of the output dim

F32 = mybir.dt.float32
MMDT = mybir.dt.float16  # matmul operand dtype (FWL + 1 cyc/row on PE)
MMNP = np.float16

# MM issue order within a step. Each output group j accumulates 5 MMs:
# an x-projection MM ('x', start=True: psum = x @ Wx[j]) plus 4 recurrent
# MMs (k=0..3). Recomputing the x MM each step seeds psum so the epilogue
# is a single fused bias+relu per group. The (j,k) suborder maximizes the
# min slack between group-k completion and the next step's first consumer
# of g'_k (slack 12 of 20 slots; >=13 provably infeasible).
STEP_ORDER = [
    (0, 'x'), (1, 'x'), (2, 'x'), (3, 'x'),
    (0, 0), (1, 0), (2, 0), (3, 0),
    (0, 1), (1, 1), (2, 1),
    (0, 2), (0, 3),
    (1, 2), (1, 3),
    (3, 1),
    (2, 2), (2, 3),
    (3, 2), (3, 3),
]


def _build_nc():
    nc = bacc.Bacc("TRN2", target_bir_lowering=False, debug=False,
                   num_devices=NCORES)
    xT = nc.dram_tensor("xT", [DIM_INPUT, B], MMDT, kind="ExternalInput")
    WhT = nc.dram_tensor("WhT", [DIM_REC, DIM_REC], MMDT, kind="ExternalInput")
    WxT = nc.dram_tensor("WxT", [DIM_INPUT, DIM_REC], MMDT, kind="ExternalInput")
    WhyT = nc.dram_tensor("WhyT", [DIM_REC, DIM_OUT], MMDT, kind="ExternalInput")
    bc = nc.dram_tensor("bc", [DIM_REC, 1], F32, kind="ExternalInput")
    by = nc.dram_tensor("by", [DIM_OUT, 1], F32, kind="ExternalInput")
    yT = nc.dram_tensor("yT", [DIM_OUT, B], F32, kind="ExternalOutput")

    RELU = mybir.ActivationFunctionType.Relu
    IDENT = mybir.ActivationFunctionType.Identity

    with TileContext(nc) as tc:
        with tc.tile_pool(name="w", bufs=1) as wp, \
             tc.tile_pool(name="s", bufs=1) as sp, \
             tc.psum_pool(name="p", bufs=1) as pp:
            wh = [wp.tile([128, DIM_REC], MMDT, name=f"wh{k}") for k in range(KJ)]
            wx = wp.tile([128, DIM_REC], MMDT, name="wx")
            why = [wp.tile([128, DIM_OUT], MMDT, name=f"why{k}") for k in range(KJ)]
            bct = [wp.tile([128, 1], F32, name=f"bct{k}") for k in range(KJ)]
            byt = [wp.tile([128, 1], F32, name=f"byt{j}") for j in range(OJ)]
            xt = sp.tile([128, B], MMDT, name="xt")
            g = [[sp.tile([128, B], MMDT, name=f"g{p}_{k}") for k in range(KJ)]
                 for p in range(2)]
            ps = [[pp.tile([128, B], F32, name=f"ps{p}_{j}") for j in range(KJ)]
                  for p in range(2)]
            psy = [ps[0][0], ps[0][1]]  # reuse phase-0 banks (free after step T-1)

            # startup DMAs spread across engine queues so the big weight
            # loads run in parallel; why/byt are only needed after the loop
            # so they trail on the sync queue.
            nc.sync.dma_start(out=xt[:], in_=xT[:])
            nc.sync.dma_start(out=wx[:], in_=WxT[:])
            for k in range(KJ):
                nc.sync.dma_start(out=bct[k][:], in_=bc[k * 128:(k + 1) * 128, :])
            nc.gpsimd.dma_start(out=wh[0][:], in_=WhT[0:128, :])
            nc.scalar.dma_start(out=wh[1][:], in_=WhT[128:256, :])
            nc.gpsimd.dma_start(out=wh[2][:], in_=WhT[256:384, :])
            nc.scalar.dma_start(out=wh[3][:], in_=WhT[384:512, :])
            for k in range(KJ):
                nc.sync.dma_start(out=why[k][:], in_=WhyT[k * 128:(k + 1) * 128, :])
            for j in range(OJ):
                nc.sync.dma_start(out=byt[j][:], in_=by[j * 128:(j + 1) * 128, :])

            ADD = mybir.AluOpType.add
            MAX = mybir.AluOpType.max

            def epilogue(dst, psrc):
                # dst_j = relu(psum_j + bc_j). Pairing (0,2) on ScalarE and
                # (1,3) on DVE staggers each engine's two ops so the second
                # op's queue wait does not extend the critical relu chain.
                nc.scalar.activation(dst[0][:], psrc[0][:], RELU,
                                     bias=bct[0][:])
                nc.scalar.activation(dst[1][:], psrc[1][:], RELU,
                                     bias=bct[1][:])
                nc.vector.tensor_scalar(dst[2][:], psrc[2][:],
                                        bct[2][:], 0.0, ADD, MAX)
                nc.vector.tensor_scalar(dst[3][:], psrc[3][:],
                                        bct[3][:], 0.0, ADD, MAX)

            # step 1 (h0 = 0): g0_j = relu((x @ W_x2h.T).T[j] + bc[j])
            for j in range(KJ):
                nc.tensor.matmul(ps[0][j][:], wx[:, j * 128:(j + 1) * 128],
                                 xt[:], start=True, stop=True)
            epilogue(g[0], ps[0])

            # 127 recurrent steps: g' = relu(x @ Wx + Wh @ g + bc)
            for s in range(1, T):
                cur, nxt = g[(s + 1) % 2], g[s % 2]
                pcur = ps[s % 2]
                grp = [0] * KJ
                for (j, k) in STEP_ORDER:
                    if k == 'x':
                        nc.tensor.matmul(pcur[j][:],
                                         wx[:, j * 128:(j + 1) * 128],
                                         xt[:], start=True, stop=False)
                    else:
                        nc.tensor.matmul(pcur[j][:],
                                         wh[k][:, j * 128:(j + 1) * 128],
                                         cur[k][:], start=False,
                                         stop=(grp[j] == KJ - 1))
                        grp[j] += 1
                epilogue(nxt, pcur)

            gfin = g[(T - 1) % 2]
            # yT[jslice] = W_h2y[jslice] @ h.T + b_h2y[jslice]
            for j in range(OJ):
                for k in range(KJ):
                    nc.tensor.matmul(psy[j][:], why[k][:, j * 128:(j + 1) * 128],
                                     gfin[k][:], start=(k == 0), stop=(k == KJ - 1))
            ytile = [sp.tile([128, B], F32, name=f"yt{j}") for j in range(OJ)]
            nc.scalar.activation(ytile[0][:], psy[0][:], IDENT, bias=byt[0][:])
            nc.vector.tensor_scalar(ytile[1][:], psy[1][:], byt[1][:], None, ADD)
            nc.sync.dma_start(out=yT[0:128, :], in_=ytile[0][:])
            nc.gpsimd.dma_start(out=yT[128:256, :], in_=ytile[1][:])

    nc.compile()
    return nc


_NC = None
TRACE = False
TRACE_TMPDIR = None
LAST_RESULTS = None


def kernel(x, W_x2h, b_x2h, W_h2h, b_h2h, W_h2y, b_h2y):
    global _NC, LAST_RESULTS
    if _NC is None:
        _NC = _build_nc()

    x = np.asarray(x, np.float32)
    shared = {
        "WhT": np.ascontiguousarray(np.asarray(W_h2h, np.float32).T.astype(MMNP)),
        "WxT": np.ascontiguousarray(np.asarray(W_x2h, np.float32).T.astype(MMNP)),
        "WhyT": np.ascontiguousarray(np.asarray(W_h2y, np.float32).T.astype(MMNP)),
        "bc": (np.asarray(b_x2h, np.float32)
               + np.asarray(b_h2h, np.float32)).reshape(DIM_REC, 1),
        "by": np.asarray(b_h2y, np.float32).reshape(DIM_OUT, 1),
    }
    ins = []
    for i in range(NCORES):
        m = dict(shared)
        m["xT"] = np.ascontiguousarray(x[i * B:(i + 1) * B, :].T.astype(MMNP))
        ins.append(m)

    kw = {}
    if TRACE:
        kw = {"trace": True, "tmpdir": TRACE_TMPDIR}
    res = run_bass_kernel_spmd(_NC, ins, core_ids=list(range(NCORES)), **kw)
    LAST_RESULTS = res
    out = np.empty((BATCH, DIM_OUT), np.float32)
    for i in range(NCORES):
        out[i * B:(i + 1) * B, :] = res.results[i]["yT"].T
    return out
